# revision 16
# baseline (speedup 1.0000x reference)
"""Attention-GRU decoder (teacher forcing) on 8 TRN2 NeuronCores.

Strategy (v4):
  Phase 0: weights split/balanced across the three DMA rings (sync/scalar/
     gpsimd; partition-range splits for the critical ones). Recurrence +
     output weights fp8e4 (x256), x-projection fp8 (x16). Precompute
     EcT_b (f16), EncWc (fp8), GIX (f16), b_out broadcast (f16). Warm-up
     AllGather absorbs launch skew + collective setup.
  Phase 1: 31 steps. fp8 stationary weights (FWL), h kept f16. PE issue
     order per step staggers gh around the attention chain (Hproj, gh/3,
     e, gh/3, alpha-transpose, gh/3, gic) so the softmax round-trip hides
     inside gh. One 4D-broadcast STT + split tanh for the attention
     pre-activation. Softmax normalization folded into the transpose
     outer-product matmuls. Sigmoid-free gates via tanh(x/2) in fused
     STTs -> single activation-table set all phase.
  Phase 2 fully interleaved: output projection grouped per AG chunk
     (rows = (core,t,b), fp8 DoubleRow). Each chunk's matmuls + logit
     STTs + exp-accums fill per-step PE idle; its sum-exp AllReduce and
     ln/subtract/output-DMA also run inside phase 1 (chunks 0-1) or the
     short tail (chunks 2-3). Output f16, cast to f32 on host.

kernel(**inputs) takes full inputs, returns [B, T-1, V] float32.
"""
import numpy as np
import ml_dtypes

import concourse.bacc as bacc
import concourse.bass as bass
import concourse.mybir as mybir
import concourse.tile as tile
from concourse.bass_utils import run_bass_kernel_spmd

F32 = mybir.dt.float32
F16 = mybir.dt.float16
F8 = mybir.dt.float8e4
AF = mybir.ActivationFunctionType
ALU = mybir.AluOpType
DR = mybir.MatmulPerfMode.DoubleRow

B, S, H, V, Dw, T = 32, 50, 1024, 32000, 512, 32
NCORES = 8
P = 128
TS = T - 1
BC = B // NCORES
VC = V // NCORES
SP = 64
NBS = BC * SP
ROWS = TS * BC
RTOT = TS * B
KH = H // P
KG = 3 * H // P
KP = KH // 2
NV = 8
NVS = VC // NV
AG_CHUNKS = [(1, 9), (9, 17), (17, 25), (25, 32)]
SW = 256.0
ISW = 1.0 / SW
SX = 16.0
ISX = 1.0 / SX

_CACHE = {}


def _build():
    nc = bacc.Bacc("TRN2", target_bir_lowering=False, debug=False,
                   num_devices=NCORES)

    def din(name, shape, dt):
        return nc.dram_tensor(name, shape, dt, kind="ExternalInput").ap()

    enct16_d = din("enct16", [P, KH, NBS], F16)
    w1et8_d = din("w1et8", [P, KH, H], F8)
    wct8_d = din("wct8", [P, KH, 3 * H], F8)
    wxa8_d = din("wxa8", [P, 5, 3 * H], F8)
    xat16_d = din("xat16", [P, 5, P], F16)
    whht8_d = din("whht8", [P, KH, 3 * H], F8)
    w1ht8_d = din("w1ht8", [P, KH, H], F8)
    w2t16_d = din("w2t16", [P, KH], F16)
    b1t_d = din("b1t", [P, KH], F32)
    bhnrep_d = din("bhnrep", [P, KH * BC], F32)
    h0t_d = din("h0t", [P, KH * BC], F16)
    wo8_d = din("wo8", [P, KP, 2, VC], F8)
    bout16_d = din("bout16", [1, VC], F16)
    out_d = nc.dram_tensor("out", [RTOT, VC], F16, kind="ExternalOutput").ap()
    outv = out_d.rearrange("(m r) v -> m r v", m=NCORES)

    rg = [list(range(NCORES))]

    with tile.TileContext(nc) as tc:
        with tc.tile_pool(name="dram", bufs=1, space="DRAM") as dram:
            agw_in = dram.tile([P, 1], F32, name="agw_in")
            agw_out = dram.tile([NCORES, P, 1], F32, name="agw_out")
            agin, agout = [], []
            for j, (tlo, thi) in enumerate(AG_CHUNKS):
                w = (thi - tlo) * BC
                agin.append(dram.tile([H, w], F16, name=f"agin{j}"))
                agout.append(dram.tile([NCORES, H, w], F16, name=f"agout{j}"))
            arinj = [dram.tile([2 * P, 1], F32, name=f"arin{j}")
                     for j in range(4)]
            aroutj = [dram.tile([2 * P, 1], F32, name=f"arout{j}")
                      for j in range(4)]

            pwo_cm = tc.tile_pool(name="pwo", bufs=1)
            pwo = pwo_cm.__enter__()
            wo8 = pwo.tile([P, KP, 2, VC], F8)
            hgaj = [pwo.tile([P, KP, 2, 2 * P], F8, name=f"hgaj{j}")
                    for j in range(4)]
            lgj = [pwo.tile([P, VC], F16, name=f"lg{i}") for i in range(8)]
            sumsj = [pwo.tile([P, NV], F32, name=f"sums{i}") for i in range(8)]
            boutrep = pwo.tile([P, VC], F16)

            with tc.tile_pool(name="pw", bufs=1) as pw:
                whht8 = pw.tile([P, KH, 3 * H], F8)
                w1ht8 = pw.tile([P, KH, H], F8)
                ecT = pw.tile([P, KH, NBS], F16)
                encwc8 = pw.tile([P, 2, 3 * H], F8)
                gixt = pw.tile([P, KG, TS, BC], F16)
                hallT = pw.tile([P, KH, T, BC], F16)
                aw16 = pw.tile([P, KH, NBS], F16)
                w2t16 = pw.tile([P, KH], F16)
                b1t = pw.tile([P, KH], F32)
                bhnrep = pw.tile([P, KH, BC], F32)
                bd1 = pw.tile([P, BC], F16)
                bd2 = pw.tile([P, BC], F16)
                warm = pw.tile([P, 1], F32)

                nc.vector.memset(warm[:], 0.0)
                nc.sync.dma_start(out=agw_in[:], in_=warm[:])
                nc.gpsimd.collective_compute(
                    "AllGather", ALU.bypass, replica_groups=rg,
                    ins=[agw_in.opt()], outs=[agw_out.opt()])

                nc.scalar.dma_start(out=w2t16[:], in_=w2t16_d[:])
                nc.scalar.dma_start(out=b1t[:], in_=b1t_d[:])
                nc.scalar.dma_start(
                    out=bhnrep[:],
                    in_=bhnrep_d[:].rearrange("p (k b) -> p k b", b=BC))
                nc.scalar.dma_start(
                    out=hallT[:, :, 0, :],
                    in_=h0t_d[:].rearrange("p (k b) -> p k b", b=BC))
                nc.vector.memset(bd1[:], 0.0)
                nc.vector.memset(bd2[:], 0.0)
                for j in range(4):
                    nc.vector.memset(hgaj[j][:], 0.0)

                # ---------------- phase 0 ----------------
                with tc.tile_pool(name="p0a", bufs=1) as p0a:
                    w1et8 = p0a.tile([P, KH, H], F8)
                    enct16 = p0a.tile([P, KH, NBS], F16)
                    nc.scalar.dma_start(out=enct16[:], in_=enct16_d[:])
                    nc.scalar.dma_start(out=w1et8[:], in_=w1et8_d[:])
                    nc.scalar.dma_start(out=w1ht8[:], in_=w1ht8_d[:])
                    # whht8 split scalar/gpsimd; wo8 on gpsimd
                    nc.scalar.dma_start(out=whht8[0:48, :, :],
                                        in_=whht8_d[0:48, :, :])
                    nc.gpsimd.dma_start(out=whht8[48:P, :, :],
                                        in_=whht8_d[48:P, :, :])
                    nc.gpsimd.dma_start(out=wo8[:], in_=wo8_d[:])

                    # GIX (fp8 x16)
                    with (
                        tc.tile_pool(name="p0gix", bufs=1) as p0gix,
                        tc.tile_pool(name="ps_gx", bufs=1,
                                     space="PSUM") as psgx,
                    ):
                        xat16 = p0gix.tile([P, 5, P], F16)
                        wxa8 = p0gix.tile([P, 5, 3 * H], F8)
                        nc.sync.dma_start(out=xat16[:], in_=xat16_d[:])
                        # wxa8 split across all three rings
                        nc.sync.dma_start(out=wxa8[0:48, :, :],
                                          in_=wxa8_d[0:48, :, :])
                        nc.scalar.dma_start(out=wxa8[48:96, :, :],
                                            in_=wxa8_d[48:96, :, :])
                        nc.gpsimd.dma_start(out=wxa8[96:P, :, :],
                                            in_=wxa8_d[96:P, :, :])
                        ps_gx = [psgx.tile([P, 4, P], F32, name=f"ps_gx{g}")
                                 for g in range(6)]
                        for k in range(5):
                            for mo in range(KG):
                                nc.tensor.matmul(
                                    ps_gx[mo // 4][:, mo % 4, :],
                                    wxa8[:, k, mo * P:(mo + 1) * P],
                                    xat16[:, k, :], start=(k == 0),
                                    stop=(k == 4))
                        for mo in range(KG):
                            nc.scalar.activation(
                                gixt[:, mo, :, :],
                                ps_gx[mo // 4][:, mo % 4, 0:ROWS].rearrange(
                                    "p (t b) -> p t b", b=BC),
                                AF.Copy, scale=ISX)

                    # EcT_b
                    with tc.tile_pool(name="ps_ec", bufs=1,
                                      space="PSUM") as psec:
                        ps_ec = [psec.tile([P, NBS], F32, name=f"ps_ec{mo}")
                                 for mo in range(KH)]
                        for k in range(KH):
                            for mo in range(KH):
                                nc.tensor.matmul(
                                    ps_ec[mo][:],
                                    w1et8[:, k, mo * P:(mo + 1) * P],
                                    enct16[:, k, :],
                                    start=(k == 0), stop=(k == KH - 1))
                        for mo in range(KH):
                            nc.scalar.activation(
                                ecT[:, mo, :], ps_ec[mo][:], AF.Identity,
                                bias=b1t[:, mo:mo + 1], scale=ISW)

                    # EncWc, Wc streamed in 6 chunks on the sync ring
                    with (
                        tc.tile_pool(name="p0ew", bufs=2) as p0ew,
                        tc.tile_pool(name="ps_ew", bufs=2,
                                     space="PSUM") as psew,
                    ):
                        for n in range(6):
                            nsl = slice(n * 512, (n + 1) * 512)
                            wcs = p0ew.tile([P, KH, 512], F8, name="wcs",
                                            tag="wcs")
                            nc.sync.dma_start(out=wcs[:],
                                              in_=wct8_d[:, :, nsl])
                            for mt in range(2):
                                ps_ew = psew.tile([P, 512], F32, name="ps_ew",
                                                  tag="ps_ew")
                                for k in range(KH):
                                    nc.tensor.matmul(
                                        ps_ew[:],
                                        enct16[:, k, mt * P:(mt + 1) * P],
                                        wcs[:, k, :],
                                        start=(k == 0), stop=(k == KH - 1))
                                nc.scalar.activation(
                                    encwc8[:, mt, nsl], ps_ew[:], AF.Copy,
                                    scale=ISW)

                    # b_out broadcast via K=1 ones matmuls
                    with (
                        tc.tile_pool(name="p0bt", bufs=1) as p0bt,
                        tc.tile_pool(name="ps_b", bufs=2,
                                     space="PSUM") as psb,
                    ):
                        bout16 = p0bt.tile([1, VC], F16)
                        nc.scalar.dma_start(out=bout16[:], in_=bout16_d[:])
                        ones16 = p0bt.tile([1, P], F16)
                        nc.vector.memset(ones16[:], 1.0)
                        for n in range(NV):
                            ps_b = psb.tile([P, NVS], F32, name="ps_b",
                                            tag="ps_b")
                            nc.tensor.matmul(
                                ps_b[:], ones16[:],
                                bout16[:, n * NVS:(n + 1) * NVS],
                                start=True, stop=True)
                            nc.scalar.copy(
                                boutrep[:, n * NVS:(n + 1) * NVS], ps_b[:])

                # phase-2 group schedule: release j at thi+4; fins for j0/j1
                # inside phase 1
                sched = {}

                def spread(j, t0s, counts):
                    g = 0
                    for i, cnt in enumerate(counts):
                        sched.setdefault(t0s + i, []).extend(
                            (j, x) for x in range(g, min(g + cnt, 16)))
                        g += cnt
                spread(0, 13, [4, 3, 3, 3, 3])
                spread(1, 21, [4, 3, 3, 3, 3])
                spread(2, 27, [4, 3, 3, 3, 3])
                FIN_AT = {21: 0, 29: 1}

                dqs = [nc.sync, nc.scalar, nc.gpsimd]

                # ------------- phase 1 + interleaved phase 2 -------------
                with (
                    tc.tile_pool(name="p1", bufs=2) as p1,
                    tc.tile_pool(name="p3", bufs=2) as p3,
                    tc.tile_pool(name="ps_hp", bufs=1, space="PSUM") as pshp,
                    tc.tile_pool(name="ps_gh", bufs=1, space="PSUM") as psgh,
                    tc.tile_pool(name="ps_gic", bufs=1, space="PSUM") as psgic,
                    tc.tile_pool(name="ps_e", bufs=1, space="PSUM") as pse,
                    tc.tile_pool(name="ps_a", bufs=1, space="PSUM") as psa,
                    tc.tile_pool(name="ps_i", bufs=2, space="PSUM") as psi,
                ):
                    def emit_p2_group(j, g):
                        half, n = divmod(g, NV)
                        M = P if j < 3 else 112
                        li = 2 * j + half
                        nsl = slice(n * NVS, (n + 1) * NVS)
                        ps_o = psi.tile([P, NVS], F32, name="ps_o", tag="psi")
                        for kp in range(KP):
                            nc.tensor.matmul(
                                ps_o[0:M, :],
                                hgaj[j][:, kp, :, half * M:(half + 1) * M],
                                wo8[:, kp, :, nsl], perf_mode=DR,
                                start=(kp == 0), stop=(kp == KP - 1))
                        nc.vector.scalar_tensor_tensor(
                            lgj[li][0:M, nsl], ps_o[0:M, :], ISW,
                            boutrep[0:M, nsl], op0=ALU.mult, op1=ALU.add)
                        et = p1.tile([P, NVS], F16, name="et", tag="et")
                        nc.scalar.activation(
                            et[0:M, :], lgj[li][0:M, nsl], AF.Exp,
                            accum_out=sumsj[li][0:M, n:n + 1])

                    def finish_j_sums(j):
                        for half in range(2):
                            M = P if j < 3 else 112
                            li = 2 * j + half
                            ssum = p1.tile([P, 1], F32, name="ssum",
                                           tag="ssum")
                            nc.vector.reduce_sum(
                                ssum[0:M, :], sumsj[li][0:M, :],
                                axis=mybir.AxisListType.X)
                            nc.sync.dma_start(
                                out=arinj[j][half * P:half * P + M, :],
                                in_=ssum[0:M, :])
                        nc.gpsimd.collective_compute(
                            "AllReduce", ALU.add, replica_groups=rg,
                            ins=[arinj[j].opt()], outs=[aroutj[j].opt()])

                    def finalize_j(j):
                        tlo, thi = AG_CHUNKS[j]
                        w_j = (thi - tlo) * BC
                        base = (tlo - 1) * BC
                        for half in range(2):
                            M = P if j < 3 else 112
                            li = 2 * j + half
                            lz = p3.tile([P, 1], F32, name="lz", tag="lz")
                            nc.scalar.dma_start(
                                out=lz[0:M, :],
                                in_=aroutj[j][half * P:half * P + M, :])
                            lzl = p3.tile([P, 1], F32, name="lzl", tag="lzl")
                            nc.scalar.activation(lzl[0:M, :], lz[0:M, :],
                                                 AF.Ln, scale=1.0)
                            ostage = p3.tile([P, VC], F16, name="ostage",
                                             tag="os")
                            hv = VC // 2
                            for hh in range(2):
                                hsl = slice(hh * hv, (hh + 1) * hv)
                                nc.vector.tensor_scalar(
                                    ostage[0:M, hsl], lgj[li][0:M, hsl],
                                    lzl[0:M, 0:1], None, op0=ALU.subtract)
                            for mr in range(4):
                                dqs[(4 * half + mr) % 3].dma_start(
                                    out=outv[4 * half + mr,
                                             base:base + w_j, :],
                                    in_=ostage[mr * w_j:(mr + 1) * w_j, :])

                    for t in range(1, T):
                        hprev = hallT[:, :, t - 1, :]

                        # Hproj in two halves (finer deps for split aw)
                        ps_hp0 = pshp.tile([P, 4, BC], F32, name="ps_hp0",
                                           tag="hp0")
                        ps_hp1 = pshp.tile([P, 4, BC], F32, name="ps_hp1",
                                           tag="hp1")
                        for mo in range(KH):
                            pst = ps_hp0 if mo < 4 else ps_hp1
                            for k in range(KH):
                                nc.tensor.matmul(
                                    pst[:, mo % 4, :],
                                    w1ht8[:, k, mo * P:(mo + 1) * P],
                                    hallT[:, k, t - 1, :],
                                    start=(k == 0), stop=(k == KH - 1))

                        ps_gh = psgh.tile([P, KG, BC], F32, name="ps_gh",
                                          tag="gh")

                        def gh_part(mlo, mhi):
                            for mo in range(mlo, mhi):
                                for k in range(KH):
                                    nc.tensor.matmul(
                                        ps_gh[:, mo, :],
                                        whht8[:, k, mo * P:(mo + 1) * P],
                                        hallT[:, k, t - 1, :],
                                        start=(k == 0), stop=(k == KH - 1))

                        gh_part(0, 8)

                        # aw in two halves (DVE STT + ACT tanh pipeline)
                        for hh, pst in ((0, ps_hp0), (1, ps_hp1)):
                            sl = slice(4 * hh, 4 * hh + 4)
                            awp = p1.tile([P, 4, BC, SP], F16,
                                          name=f"awp{hh}", tag=f"awp{hh}")
                            nc.vector.scalar_tensor_tensor(
                                awp[:],
                                pst[:].broadcast_to([P, 4, BC, SP]),
                                ISW,
                                ecT[:, sl, :].rearrange(
                                    "p k (b s) -> p k b s", s=SP),
                                op0=ALU.mult, op1=ALU.add)
                            nc.scalar.activation(
                                aw16[:, sl, :].rearrange(
                                    "p k (b s) -> p k b s", s=SP),
                                awp[:], AF.Tanh)

                        # e = w2 . aw
                        ps_e = pse.tile([1, NBS], F32, name="ps_e", tag="e")
                        for k in range(KH):
                            nc.tensor.matmul(
                                ps_e[:], w2t16[:, k:k + 1], aw16[:, k, :],
                                start=(k == 0), stop=(k == KH - 1))

                        gh_part(8, 16)

                        # softmax (normalization folded into transpose)
                        expu = p1.tile([1, NBS], F16, name="expu", tag="expu")
                        nc.scalar.activation(expu[:], ps_e[:], AF.Exp)
                        s4 = p1.tile([1, BC], F32, name="s4", tag="s4")
                        nc.vector.reduce_sum(
                            s4[:], expu[:].rearrange("a (b s) -> a b s", s=SP)
                            [:, :, 0:S],
                            axis=mybir.AxisListType.X)
                        r4 = p1.tile([1, BC], F32, name="r4", tag="r4")
                        nc.vector.reciprocal(r4[:], s4[:])
                        r4h = p1.tile([1, BC], F16, name="r4h", tag="r4h")
                        nc.vector.tensor_copy(r4h[:], r4[:])

                        ps_a = psa.tile([P, 4], F32, name="ps_a", tag="a1")
                        nc.tensor.matmul(ps_a[:, 0:2], expu[:, 0:P],
                                         r4h[:, 0:2], start=True, stop=True)
                        nc.tensor.matmul(ps_a[:, 2:4], expu[:, P:NBS],
                                         r4h[:, 2:4], start=True, stop=True)

                        gh_part(16, KG)

                        nc.vector.tensor_copy(bd1[0:64, 0:1], ps_a[0:64, 0:1])
                        nc.vector.tensor_copy(bd1[64:P, 1:2], ps_a[64:P, 1:2])
                        nc.vector.tensor_copy(bd2[0:64, 2:3], ps_a[0:64, 2:3])
                        nc.vector.tensor_copy(bd2[64:P, 3:4], ps_a[64:P, 3:4])

                        ps_gic = psgic.tile([P, KG, BC], F32, name="ps_gic",
                                            tag="gic")
                        for mo in range(KG):
                            nc.tensor.matmul(
                                ps_gic[:, mo, :],
                                encwc8[:, 0, mo * P:(mo + 1) * P],
                                bd1[:], start=True, stop=False)
                            nc.tensor.matmul(
                                ps_gic[:, mo, :],
                                encwc8[:, 1, mo * P:(mo + 1) * P],
                                bd2[:], start=False, stop=True)

                        # gates
                        s1 = p1.tile([P, KG, BC], F32, name="s1", tag="s1")
                        nc.vector.tensor_add(s1[:], ps_gic[:],
                                             gixt[:, :, t - 1, :])
                        b2 = p1.tile([P, 2 * KH, BC], F32, name="b2", tag="b2")
                        nc.vector.scalar_tensor_tensor(
                            b2[:], ps_gh[:, 0:2 * KH, :], ISW,
                            s1[:, 0:2 * KH, :], op0=ALU.mult, op1=ALU.add)
                        tt = p1.tile([P, 2 * KH, BC], F32, name="tt", tag="tt")
                        nc.scalar.activation(tt[:], b2[:], AF.Tanh, scale=0.5)
                        hn = p1.tile([P, KH, BC], F32, name="hn", tag="hn")
                        nc.vector.scalar_tensor_tensor(
                            hn[:], ps_gh[:, 2 * KH:KG, :], ISW, bhnrep[:],
                            op0=ALU.mult, op1=ALU.add)
                        m1p = p1.tile([P, KH, BC], F32, name="m1p", tag="m1p")
                        nc.vector.scalar_tensor_tensor(
                            m1p[:], tt[:, 0:KH, :], 1.0, hn[:],
                            op0=ALU.add, op1=ALU.mult)
                        s3 = p1.tile([P, KH, BC], F32, name="s3", tag="s3")
                        nc.vector.scalar_tensor_tensor(
                            s3[:], m1p[:], 0.5, s1[:, 2 * KH:KG, :],
                            op0=ALU.mult, op1=ALU.add)
                        nn_t = p1.tile([P, KH, BC], F32, name="nn_t", tag="nn")
                        nc.scalar.activation(nn_t[:], s3[:], AF.Tanh)
                        dd = p1.tile([P, KH, BC], F32, name="dd", tag="dd")
                        nc.vector.tensor_sub(dd[:], hprev, nn_t[:])
                        e1 = p1.tile([P, KH, BC], F32, name="e1", tag="e1")
                        nc.vector.scalar_tensor_tensor(
                            e1[:], tt[:, KH:2 * KH, :], 1.0, dd[:],
                            op0=ALU.add, op1=ALU.mult)
                        nc.vector.scalar_tensor_tensor(
                            hallT[:, :, t, :], e1[:], 0.5, nn_t[:],
                            op0=ALU.mult, op1=ALU.add)

                        # interleaved phase-2 groups: PE work lands after
                        # gic; their DVE/ACT ops queue behind the gates so
                        # they never delay the recurrence chain
                        for (j, g) in sched.get(t, []):
                            emit_p2_group(j, g)
                            if g == 15:
                                finish_j_sums(j)

                        # early finalize for chunks whose AllReduce is done
                        if t in FIN_AT:
                            finalize_j(FIN_AT[t])

                        # partial allgather of finished h slots (single DMAs)
                        for j, (tlo, thi) in enumerate(AG_CHUNKS):
                            if t == thi - 1:
                                w_j = (thi - tlo) * BC
                                nc.sync.dma_start(
                                    out=agin[j][:].rearrange(
                                        "(k p) w -> p k w", p=P),
                                    in_=hallT[:, :, tlo:thi, :])
                                nc.gpsimd.collective_compute(
                                    "AllGather", ALU.bypass,
                                    replica_groups=rg,
                                    ins=[agin[j].opt()],
                                    outs=[agout[j].opt()])
                                hgs = p1.tile([P, NCORES, KH, 32], F16,
                                              name="hgs", tag="hgs")
                                nc.scalar.dma_start(
                                    out=hgs[:].rearrange(
                                        "p r k w -> p (r k) w")[:, :, 0:w_j],
                                    in_=agout[j][:].rearrange(
                                        "r (k p) w -> p (r k) w", p=P))
                                for kp in range(KP):
                                    nc.vector.tensor_copy(
                                        hgaj[j][:, kp, :, 0:NCORES * w_j]
                                        .rearrange("p a (m w) -> p a m w",
                                                   w=w_j),
                                        hgs[:, :, 2 * kp:2 * kp + 2, 0:w_j]
                                        .rearrange("p m a w -> p a m w"))

                    # tail: last chunk compute + remaining finalizes
                    for g in range(16):
                        emit_p2_group(3, g)
                    finish_j_sums(3)
                    finalize_j(2)
                    finalize_j(3)

            pwo_cm.__exit__(None, None, None)

    nc.compile()
    return nc


def _t8(w, nk=8):
    m = w.shape[1]
    return np.ascontiguousarray(w.reshape(nk, P, m).transpose(1, 0, 2))


def _f8(x):
    return np.clip(x, -240.0, 240.0).astype(ml_dtypes.float8_e4m3)


def _prep_inputs(inputs):
    enc = np.asarray(inputs["encoder_outputs"], np.float32)
    ehid = np.asarray(inputs["encoder_hidden"], np.float32)
    targets = np.asarray(inputs["targets"])
    emb = np.asarray(inputs["emb"], np.float32)
    W1 = np.asarray(inputs["attn_W1"], np.float32)
    b1 = np.asarray(inputs["attn_b1"], np.float32)
    W2 = np.asarray(inputs["attn_W2"], np.float32)
    W_ih = np.asarray(inputs["W_ih"], np.float32)
    b_ih = np.asarray(inputs["b_ih"], np.float32)
    W_hh = np.asarray(inputs["W_hh"], np.float32)
    b_hh = np.asarray(inputs["b_hh"], np.float32)
    W_out = np.asarray(inputs["W_out"], np.float32)
    b_out = np.asarray(inputs["b_out"], np.float32)

    w1et8 = _f8(_t8(W1[:, :H].T) * SW)
    w1ht8 = _f8(_t8(np.ascontiguousarray(W1[:, H:]).T) * SW)
    wct8 = _f8(_t8(np.ascontiguousarray(W_ih[:, Dw:]).T) * SW)
    whht8 = _f8(_t8(W_hh.T) * SW)
    wxa = np.zeros((640, 3 * H), np.float32)
    wxa[:Dw] = W_ih[:, :Dw].T
    wxa[Dw] = b_ih + np.concatenate([b_hh[:2 * H], np.zeros(H, np.float32)])
    wxa8 = _f8(_t8(wxa, nk=5) * SX)
    w2t16 = np.ascontiguousarray(W2[0].reshape(KH, P).T).astype(np.float16)
    b1t = np.ascontiguousarray(b1.reshape(KH, P).T)
    bhnrep = np.ascontiguousarray(
        np.repeat(b_hh[2 * H:].reshape(KH, P).T[:, :, None], BC, axis=2)
        .reshape(P, KH * BC))

    x_all = emb[targets[:, :TS]]

    in_maps = []
    for c in range(NCORES):
        bsl = slice(c * BC, (c + 1) * BC)
        vsl = slice(c * VC, (c + 1) * VC)
        encT = np.zeros((H, BC, SP), np.float32)
        encT[:, :, :S] = enc[bsl].transpose(2, 0, 1)
        enct16 = _t8(encT.reshape(H, NBS)).astype(np.float16)
        xat = np.zeros((640, P), np.float32)
        xat[:Dw, :ROWS] = x_all[bsl].transpose(2, 1, 0).reshape(Dw, ROWS)
        xat[Dw, :ROWS] = 1.0
        xat16 = _t8(xat, nk=5).astype(np.float16)
        h0t = np.ascontiguousarray(
            ehid[0, bsl].T.reshape(KH, P, BC).transpose(1, 0, 2)
            .reshape(P, KH * BC)).astype(np.float16)
        wo8 = _f8(np.ascontiguousarray(
            (W_out[vsl].T * SW).reshape(KP, 2, P, VC).transpose(2, 0, 1, 3)))
        bout16 = np.ascontiguousarray(b_out[vsl][None, :]).astype(np.float16)
        in_maps.append({
            "enct16": enct16, "w1et8": w1et8, "wct8": wct8,
            "wxa8": wxa8, "xat16": xat16, "whht8": whht8, "w1ht8": w1ht8,
            "w2t16": w2t16, "b1t": b1t, "bhnrep": bhnrep, "h0t": h0t,
            "wo8": wo8, "bout16": bout16,
        })
    return in_maps


def kernel(**inputs):
    if "nc" not in _CACHE:
        _CACHE["nc"] = _build()
    nc = _CACHE["nc"]
    in_maps = _prep_inputs(inputs)
    res = run_bass_kernel_spmd(nc, in_maps, core_ids=list(range(NCORES)))
    L = np.stack([np.asarray(res.results[c]["out"], np.float32)
                  for c in range(NCORES)])
    L = (L.reshape(NCORES, NCORES, TS, BC, VC)
         .transpose(1, 3, 2, 0, 4).reshape(B, TS, V))
    return np.ascontiguousarray(L, dtype=np.float32)


# revision 20
# speedup vs baseline: 1.2004x; 1.2004x over previous
"""Attention-GRU decoder (teacher forcing) on 8 TRN2 NeuronCores.

Strategy (v4):
  Phase 0: weights split/balanced across the three DMA rings (sync/scalar/
     gpsimd; partition-range splits for the critical ones). Recurrence +
     output weights fp8e4 (x256), x-projection fp8 (x16). Precompute
     EcT_b (f16), EncWc (fp8), GIX (f16), b_out broadcast (f16). Warm-up
     AllGather absorbs launch skew + collective setup.
  Phase 1: 31 steps. fp8 stationary weights (FWL), h kept f16. PE issue
     order per step staggers gh around the attention chain (Hproj, gh/3,
     e, gh/3, alpha-transpose, gh/3, gic) so the softmax round-trip hides
     inside gh. One 4D-broadcast STT + split tanh for the attention
     pre-activation. Softmax normalization folded into the transpose
     outer-product matmuls. Sigmoid-free gates via tanh(x/2) in fused
     STTs -> single activation-table set all phase.
  Phase 2 fully interleaved: output projection grouped per AG chunk
     (rows = (core,t,b), fp8 DoubleRow). Each chunk's matmuls + logit
     STTs + exp-accums fill per-step PE idle; its sum-exp AllReduce and
     ln/subtract/output-DMA also run inside phase 1 (chunks 0-1) or the
     short tail (chunks 2-3). Output f16, cast to f32 on host.

kernel(**inputs) takes full inputs, returns [B, T-1, V] float32.
"""
import numpy as np
import ml_dtypes

import concourse.bacc as bacc
import concourse.bass as bass
import concourse.mybir as mybir
import concourse.tile as tile
from concourse.bass_utils import run_bass_kernel_spmd

F32 = mybir.dt.float32
F16 = mybir.dt.float16
F8 = mybir.dt.float8e4
AF = mybir.ActivationFunctionType
ALU = mybir.AluOpType
DR = mybir.MatmulPerfMode.DoubleRow

B, S, H, V, Dw, T = 32, 50, 1024, 32000, 512, 32
NCORES = 8
P = 128
TS = T - 1
BC = B // NCORES
VC = V // NCORES
SP = 64
NBS = BC * SP
ROWS = TS * BC
RTOT = TS * B
KH = H // P
KG = 3 * H // P
KP = KH // 2
NV = 8
NVS = VC // NV
AG_CHUNKS = [(1, 9), (9, 17), (17, 25), (25, 32)]
SW = 256.0
ISW = 1.0 / SW
SX = 16.0
ISX = 1.0 / SX

_CACHE = {}


def _build():
    nc = bacc.Bacc("TRN2", target_bir_lowering=False, debug=False,
                   num_devices=NCORES)

    def din(name, shape, dt):
        return nc.dram_tensor(name, shape, dt, kind="ExternalInput").ap()

    enct16_d = din("enct16", [P, KH, NBS], F16)
    w1et8_d = din("w1et8", [P, KH, H], F8)
    wct8_d = din("wct8", [P, KH, 3 * H], F8)
    wxa8_d = din("wxa8", [P, 5, 3 * H], F8)
    xat16_d = din("xat16", [P, 5, P], F16)
    whht8_d = din("whht8", [P, KH, 3 * H], F8)
    w1ht8_d = din("w1ht8", [P, KH, H], F8)
    w2t16_d = din("w2t16", [P, KH], F16)
    b1t_d = din("b1t", [P, KH], F32)
    bhnrep_d = din("bhnrep", [P, KH * BC], F32)
    h0t_d = din("h0t", [P, KH * BC], F16)
    wo8_d = din("wo8", [P, KP, 2, VC], F8)
    bout16_d = din("bout16", [1, VC], F16)
    out_d = nc.dram_tensor("out", [RTOT, VC], F16, kind="ExternalOutput").ap()
    outv = out_d.rearrange("(m r) v -> m r v", m=NCORES)

    rg = [list(range(NCORES))]

    with tile.TileContext(nc) as tc:
        with tc.tile_pool(name="dram", bufs=1, space="DRAM") as dram:
            agw_in = dram.tile([P, 1], F32, name="agw_in")
            agw_out = dram.tile([NCORES, P, 1], F32, name="agw_out")
            agin, agout = [], []
            for j, (tlo, thi) in enumerate(AG_CHUNKS):
                w = (thi - tlo) * BC
                agin.append(dram.tile([H, w], F16, name=f"agin{j}"))
                agout.append(dram.tile([NCORES, H, w], F16, name=f"agout{j}"))
            arinj = [dram.tile([2 * P, 1], F32, name=f"arin{j}")
                     for j in range(4)]
            aroutj = [dram.tile([2 * P, 1], F32, name=f"arout{j}")
                      for j in range(4)]

            pwo_cm = tc.tile_pool(name="pwo", bufs=1)
            pwo = pwo_cm.__enter__()
            wo8 = pwo.tile([P, KP, 2, VC], F8)
            hgaj = [pwo.tile([P, KP, 2, 2 * P], F8, name=f"hgaj{j}")
                    for j in range(4)]
            lgj = [pwo.tile([P, VC], F16, name=f"lg{i}") for i in range(8)]
            sumsj = [pwo.tile([P, NV], F32, name=f"sums{i}") for i in range(8)]
            boutrep = pwo.tile([P, VC], F16)

            with tc.tile_pool(name="pw", bufs=1) as pw:
                whht8 = pw.tile([P, KH, 3 * H], F8)
                w1ht8 = pw.tile([P, KH, H], F8)
                ecT = pw.tile([P, KH, NBS], F16)
                encwc8 = pw.tile([P, 2, 3 * H], F8)
                gixt = pw.tile([P, KG, TS, BC], F16)
                hallT = pw.tile([P, KH, T, BC], F16)
                aw16 = pw.tile([P, KH, NBS], F16)
                w2t16 = pw.tile([P, KH], F16)
                b1t = pw.tile([P, KH], F32)
                bhnrep = pw.tile([P, KH, BC], F32)
                bd1 = pw.tile([P, BC], F16)
                bd2 = pw.tile([P, BC], F16)
                warm = pw.tile([P, 1], F32)

                nc.vector.memset(warm[:], 0.0)
                nc.sync.dma_start(out=agw_in[:], in_=warm[:])
                nc.gpsimd.collective_compute(
                    "AllGather", ALU.bypass, replica_groups=rg,
                    ins=[agw_in.opt()], outs=[agw_out.opt()])

                nc.scalar.dma_start(out=w2t16[:], in_=w2t16_d[:])
                nc.scalar.dma_start(out=b1t[:], in_=b1t_d[:])
                nc.scalar.dma_start(
                    out=bhnrep[:],
                    in_=bhnrep_d[:].rearrange("p (k b) -> p k b", b=BC))
                nc.scalar.dma_start(
                    out=hallT[:, :, 0, :],
                    in_=h0t_d[:].rearrange("p (k b) -> p k b", b=BC))
                nc.vector.memset(bd1[:], 0.0)
                nc.vector.memset(bd2[:], 0.0)
                for j in range(4):
                    nc.vector.memset(hgaj[j][:], 0.0)

                # ---------------- phase 0 ----------------
                with tc.tile_pool(name="p0a", bufs=1) as p0a:
                    w1et8 = p0a.tile([P, KH, H], F8)
                    enct16 = p0a.tile([P, KH, NBS], F16)

                    # GIX (fp8 x16)
                    with (
                        tc.tile_pool(name="p0gix", bufs=1) as p0gix,
                        tc.tile_pool(name="ps_gx", bufs=1,
                                     space="PSUM") as psgx,
                    ):
                        xat16 = p0gix.tile([P, 5, P], F16)
                        wxa8 = p0gix.tile([P, 5, 3 * H], F8)
                        # GIX-critical first on every ring
                        nc.sync.dma_start(out=xat16[:], in_=xat16_d[:])
                        nc.sync.dma_start(out=wxa8[0:48, :, :],
                                          in_=wxa8_d[0:48, :, :])
                        nc.scalar.dma_start(out=wxa8[48:96, :, :],
                                            in_=wxa8_d[48:96, :, :])
                        nc.gpsimd.dma_start(out=wxa8[96:P, :, :],
                                            in_=wxa8_d[96:P, :, :])
                        # the rest behind them
                        nc.scalar.dma_start(out=enct16[:], in_=enct16_d[:])
                        nc.scalar.dma_start(out=w1et8[:], in_=w1et8_d[:])
                        nc.scalar.dma_start(out=w1ht8[:], in_=w1ht8_d[:])
                        nc.scalar.dma_start(out=whht8[0:48, :, :],
                                            in_=whht8_d[0:48, :, :])
                        nc.gpsimd.dma_start(out=whht8[48:P, :, :],
                                            in_=whht8_d[48:P, :, :])
                        nc.gpsimd.dma_start(out=wo8[:], in_=wo8_d[:])
                        ps_gx = [psgx.tile([P, 4, P], F32, name=f"ps_gx{g}")
                                 for g in range(6)]
                        for k in range(5):
                            for mo in range(KG):
                                nc.tensor.matmul(
                                    ps_gx[mo // 4][:, mo % 4, :],
                                    wxa8[:, k, mo * P:(mo + 1) * P],
                                    xat16[:, k, :], start=(k == 0),
                                    stop=(k == 4))
                        for mo in range(KG):
                            nc.scalar.activation(
                                gixt[:, mo, :, :],
                                ps_gx[mo // 4][:, mo % 4, 0:ROWS].rearrange(
                                    "p (t b) -> p t b", b=BC),
                                AF.Copy, scale=ISX)

                    # EcT_b
                    with tc.tile_pool(name="ps_ec", bufs=1,
                                      space="PSUM") as psec:
                        ps_ec = [psec.tile([P, NBS], F32, name=f"ps_ec{mo}")
                                 for mo in range(KH)]
                        for k in range(KH):
                            for mo in range(KH):
                                nc.tensor.matmul(
                                    ps_ec[mo][:],
                                    w1et8[:, k, mo * P:(mo + 1) * P],
                                    enct16[:, k, :],
                                    start=(k == 0), stop=(k == KH - 1))
                        for mo in range(KH):
                            nc.scalar.activation(
                                ecT[:, mo, :], ps_ec[mo][:], AF.Identity,
                                bias=b1t[:, mo:mo + 1], scale=ISW)

                    # EncWc, Wc streamed in 6 chunks on the sync ring
                    with (
                        tc.tile_pool(name="p0ew", bufs=2) as p0ew,
                        tc.tile_pool(name="ps_ew", bufs=2,
                                     space="PSUM") as psew,
                    ):
                        for n in range(6):
                            nsl = slice(n * 512, (n + 1) * 512)
                            wcs = p0ew.tile([P, KH, 512], F8, name="wcs",
                                            tag="wcs")
                            nc.sync.dma_start(out=wcs[:],
                                              in_=wct8_d[:, :, nsl])
                            for mt in range(2):
                                ps_ew = psew.tile([P, 512], F32, name="ps_ew",
                                                  tag="ps_ew")
                                for k in range(KH):
                                    nc.tensor.matmul(
                                        ps_ew[:],
                                        enct16[:, k, mt * P:(mt + 1) * P],
                                        wcs[:, k, :],
                                        start=(k == 0), stop=(k == KH - 1))
                                nc.scalar.activation(
                                    encwc8[:, mt, nsl], ps_ew[:], AF.Copy,
                                    scale=ISW)

                    # b_out broadcast via K=1 ones matmuls
                    with (
                        tc.tile_pool(name="p0bt", bufs=1) as p0bt,
                        tc.tile_pool(name="ps_b", bufs=2,
                                     space="PSUM") as psb,
                    ):
                        bout16 = p0bt.tile([1, VC], F16)
                        nc.scalar.dma_start(out=bout16[:], in_=bout16_d[:])
                        ones16 = p0bt.tile([1, P], F16)
                        nc.vector.memset(ones16[:], 1.0)
                        for n in range(NV):
                            ps_b = psb.tile([P, NVS], F32, name="ps_b",
                                            tag="ps_b")
                            nc.tensor.matmul(
                                ps_b[:], ones16[:],
                                bout16[:, n * NVS:(n + 1) * NVS],
                                start=True, stop=True)
                            nc.scalar.copy(
                                boutrep[:, n * NVS:(n + 1) * NVS], ps_b[:])

                # phase-2 group schedule: release j at thi+4; fins for j0/j1
                # inside phase 1
                sched = {}

                def spread(j, t0s, counts):
                    g = 0
                    for i, cnt in enumerate(counts):
                        sched.setdefault(t0s + i, []).extend(
                            (j, x) for x in range(g, min(g + cnt, 16)))
                        g += cnt
                spread(0, 13, [4, 3, 3, 3, 3])
                spread(1, 21, [4, 3, 3, 3, 3])
                spread(2, 27, [4, 4, 4, 4])
                FIN_AT = {21: 0, 29: 1}

                dqs = [nc.sync, nc.scalar, nc.gpsimd]

                # ------------- phase 1 + interleaved phase 2 -------------
                with (
                    tc.tile_pool(name="p1", bufs=2) as p1,
                    tc.tile_pool(name="p3", bufs=2) as p3,
                    tc.tile_pool(name="ps_hp", bufs=1, space="PSUM") as pshp,
                    tc.tile_pool(name="ps_gh", bufs=1, space="PSUM") as psgh,
                    tc.tile_pool(name="ps_gic", bufs=1, space="PSUM") as psgic,
                    tc.tile_pool(name="ps_e", bufs=1, space="PSUM") as pse,
                    tc.tile_pool(name="ps_a", bufs=1, space="PSUM") as psa,
                    tc.tile_pool(name="ps_i", bufs=2, space="PSUM") as psi,
                ):
                    def emit_p2_group(j, g):
                        half, n = divmod(g, NV)
                        M = P if j < 3 else 112
                        li = 2 * j + half
                        nsl = slice(n * NVS, (n + 1) * NVS)
                        ps_o = psi.tile([P, NVS], F32, name="ps_o", tag="psi")
                        for kp in range(KP):
                            nc.tensor.matmul(
                                ps_o[0:M, :],
                                hgaj[j][:, kp, :, half * M:(half + 1) * M],
                                wo8[:, kp, :, nsl], perf_mode=DR,
                                start=(kp == 0), stop=(kp == KP - 1))
                        nc.vector.scalar_tensor_tensor(
                            lgj[li][0:M, nsl], ps_o[0:M, :], ISW,
                            boutrep[0:M, nsl], op0=ALU.mult, op1=ALU.add)
                        et = p1.tile([P, NVS], F16, name="et", tag="et")
                        nc.scalar.activation(
                            et[0:M, :], lgj[li][0:M, nsl], AF.Exp,
                            accum_out=sumsj[li][0:M, n:n + 1])

                    def finish_j_sums(j):
                        for half in range(2):
                            M = P if j < 3 else 112
                            li = 2 * j + half
                            ssum = p1.tile([P, 1], F32, name="ssum",
                                           tag="ssum")
                            nc.vector.reduce_sum(
                                ssum[0:M, :], sumsj[li][0:M, :],
                                axis=mybir.AxisListType.X)
                            nc.sync.dma_start(
                                out=arinj[j][half * P:half * P + M, :],
                                in_=ssum[0:M, :])
                        nc.gpsimd.collective_compute(
                            "AllReduce", ALU.add, replica_groups=rg,
                            ins=[arinj[j].opt()], outs=[aroutj[j].opt()])

                    def finalize_j(j):
                        tlo, thi = AG_CHUNKS[j]
                        w_j = (thi - tlo) * BC
                        base = (tlo - 1) * BC
                        for half in range(2):
                            M = P if j < 3 else 112
                            li = 2 * j + half
                            lz = p3.tile([P, 1], F32, name="lz", tag="lz")
                            nc.scalar.dma_start(
                                out=lz[0:M, :],
                                in_=aroutj[j][half * P:half * P + M, :])
                            lzl = p3.tile([P, 1], F32, name="lzl", tag="lzl")
                            nc.scalar.activation(lzl[0:M, :], lz[0:M, :],
                                                 AF.Ln, scale=1.0)
                            ostage = p3.tile([P, VC], F16, name="ostage",
                                             tag="os")
                            hv = VC // 2
                            for hh in range(2):
                                hsl = slice(hh * hv, (hh + 1) * hv)
                                nc.vector.tensor_scalar(
                                    ostage[0:M, hsl], lgj[li][0:M, hsl],
                                    lzl[0:M, 0:1], None, op0=ALU.subtract)
                            for mr in range(4):
                                dqs[(4 * half + mr) % 3].dma_start(
                                    out=outv[4 * half + mr,
                                             base:base + w_j, :],
                                    in_=ostage[mr * w_j:(mr + 1) * w_j, :])

                    for t in range(1, T):
                        hprev = hallT[:, :, t - 1, :]

                        # Hproj in two halves (finer deps for split aw)
                        ps_hp0 = pshp.tile([P, 4, BC], F32, name="ps_hp0",
                                           tag="hp0")
                        ps_hp1 = pshp.tile([P, 4, BC], F32, name="ps_hp1",
                                           tag="hp1")
                        for mo in range(KH):
                            pst = ps_hp0 if mo < 4 else ps_hp1
                            for k in range(KH):
                                nc.tensor.matmul(
                                    pst[:, mo % 4, :],
                                    w1ht8[:, k, mo * P:(mo + 1) * P],
                                    hallT[:, k, t - 1, :],
                                    start=(k == 0), stop=(k == KH - 1))

                        ps_gh = psgh.tile([P, KG, BC], F32, name="ps_gh",
                                          tag="gh")

                        def gh_part(mlo, mhi):
                            for mo in range(mlo, mhi):
                                for k in range(KH):
                                    nc.tensor.matmul(
                                        ps_gh[:, mo, :],
                                        whht8[:, k, mo * P:(mo + 1) * P],
                                        hallT[:, k, t - 1, :],
                                        start=(k == 0), stop=(k == KH - 1))

                        gh_part(0, KG)

                        # aw in two halves (DVE STT + ACT tanh pipeline)
                        for hh, pst in ((0, ps_hp0), (1, ps_hp1)):
                            sl = slice(4 * hh, 4 * hh + 4)
                            awp = p1.tile([P, 4, BC, SP], F16,
                                          name=f"awp{hh}", tag=f"awp{hh}")
                            nc.vector.scalar_tensor_tensor(
                                awp[:],
                                pst[:].broadcast_to([P, 4, BC, SP]),
                                ISW,
                                ecT[:, sl, :].rearrange(
                                    "p k (b s) -> p k b s", s=SP),
                                op0=ALU.mult, op1=ALU.add)
                            nc.scalar.activation(
                                aw16[:, sl, :].rearrange(
                                    "p k (b s) -> p k b s", s=SP),
                                awp[:], AF.Tanh)

                        # e = w2 . aw
                        ps_e = pse.tile([1, NBS], F32, name="ps_e", tag="e")
                        for k in range(KH):
                            nc.tensor.matmul(
                                ps_e[:], w2t16[:, k:k + 1], aw16[:, k, :],
                                start=(k == 0), stop=(k == KH - 1))

                        # softmax (normalization folded into transpose)
                        expu = p1.tile([1, NBS], F16, name="expu", tag="expu")
                        nc.scalar.activation(expu[:], ps_e[:], AF.Exp)
                        s4 = p1.tile([1, BC], F32, name="s4", tag="s4")
                        nc.vector.reduce_sum(
                            s4[:], expu[:].rearrange("a (b s) -> a b s", s=SP)
                            [:, :, 0:S],
                            axis=mybir.AxisListType.X)
                        r4 = p1.tile([1, BC], F32, name="r4", tag="r4")
                        nc.vector.reciprocal(r4[:], s4[:])
                        r4h = p1.tile([1, BC], F16, name="r4h", tag="r4h")
                        nc.vector.tensor_copy(r4h[:], r4[:])

                        ps_a = psa.tile([P, 4], F32, name="ps_a", tag="a1")
                        nc.tensor.matmul(ps_a[:, 0:2], expu[:, 0:P],
                                         r4h[:, 0:2], start=True, stop=True)
                        nc.tensor.matmul(ps_a[:, 2:4], expu[:, P:NBS],
                                         r4h[:, 2:4], start=True, stop=True)

                        nc.vector.tensor_copy(bd1[0:64, 0:1], ps_a[0:64, 0:1])
                        nc.vector.tensor_copy(bd1[64:P, 1:2], ps_a[64:P, 1:2])
                        nc.vector.tensor_copy(bd2[0:64, 2:3], ps_a[0:64, 2:3])
                        nc.vector.tensor_copy(bd2[64:P, 3:4], ps_a[64:P, 3:4])

                        ps_gic = psgic.tile([P, KG, BC], F32, name="ps_gic",
                                            tag="gic")
                        for mo in range(KG):
                            nc.tensor.matmul(
                                ps_gic[:, mo, :],
                                encwc8[:, 0, mo * P:(mo + 1) * P],
                                bd1[:], start=True, stop=False)
                            nc.tensor.matmul(
                                ps_gic[:, mo, :],
                                encwc8[:, 1, mo * P:(mo + 1) * P],
                                bd2[:], start=False, stop=True)

                        # gates
                        s1 = p1.tile([P, KG, BC], F32, name="s1", tag="s1")
                        nc.vector.tensor_add(s1[:], ps_gic[:],
                                             gixt[:, :, t - 1, :])
                        b2 = p1.tile([P, 2 * KH, BC], F32, name="b2", tag="b2")
                        nc.vector.scalar_tensor_tensor(
                            b2[:], ps_gh[:, 0:2 * KH, :], ISW,
                            s1[:, 0:2 * KH, :], op0=ALU.mult, op1=ALU.add)
                        tt = p1.tile([P, 2 * KH, BC], F32, name="tt", tag="tt")
                        nc.scalar.activation(tt[:], b2[:], AF.Tanh, scale=0.5)
                        hn = p1.tile([P, KH, BC], F32, name="hn", tag="hn")
                        nc.vector.scalar_tensor_tensor(
                            hn[:], ps_gh[:, 2 * KH:KG, :], ISW, bhnrep[:],
                            op0=ALU.mult, op1=ALU.add)
                        m1p = p1.tile([P, KH, BC], F32, name="m1p", tag="m1p")
                        nc.vector.scalar_tensor_tensor(
                            m1p[:], tt[:, 0:KH, :], 1.0, hn[:],
                            op0=ALU.add, op1=ALU.mult)
                        s3 = p1.tile([P, KH, BC], F32, name="s3", tag="s3")
                        nc.vector.scalar_tensor_tensor(
                            s3[:], m1p[:], 0.5, s1[:, 2 * KH:KG, :],
                            op0=ALU.mult, op1=ALU.add)
                        nn_t = p1.tile([P, KH, BC], F32, name="nn_t", tag="nn")
                        nc.scalar.activation(nn_t[:], s3[:], AF.Tanh)
                        dd = p1.tile([P, KH, BC], F32, name="dd", tag="dd")
                        nc.vector.tensor_sub(dd[:], hprev, nn_t[:])
                        e1 = p1.tile([P, KH, BC], F32, name="e1", tag="e1")
                        nc.vector.scalar_tensor_tensor(
                            e1[:], tt[:, KH:2 * KH, :], 1.0, dd[:],
                            op0=ALU.add, op1=ALU.mult)
                        nc.vector.scalar_tensor_tensor(
                            hallT[:, :, t, :], e1[:], 0.5, nn_t[:],
                            op0=ALU.mult, op1=ALU.add)

                        # interleaved phase-2 groups: PE work lands after
                        # gic; their DVE/ACT ops queue behind the gates so
                        # they never delay the recurrence chain
                        for (j, g) in sched.get(t, []):
                            emit_p2_group(j, g)
                            if g == 15:
                                finish_j_sums(j)

                        # early finalize for chunks whose AllReduce is done
                        if t in FIN_AT:
                            finalize_j(FIN_AT[t])

                        # partial allgather of finished h slots (single DMAs)
                        for j, (tlo, thi) in enumerate(AG_CHUNKS):
                            if t == thi - 1:
                                w_j = (thi - tlo) * BC
                                nc.sync.dma_start(
                                    out=agin[j][:].rearrange(
                                        "(k p) w -> p k w", p=P),
                                    in_=hallT[:, :, tlo:thi, :])
                                nc.gpsimd.collective_compute(
                                    "AllGather", ALU.bypass,
                                    replica_groups=rg,
                                    ins=[agin[j].opt()],
                                    outs=[agout[j].opt()])
                                hgs = p1.tile([P, NCORES, KH, 32], F16,
                                              name="hgs", tag="hgs")
                                nc.gpsimd.dma_start(
                                    out=hgs[:].rearrange(
                                        "p r k w -> p (r k) w")[:, :, 0:w_j],
                                    in_=agout[j][:].rearrange(
                                        "r (k p) w -> p (r k) w", p=P))
                                for kp in range(KP):
                                    nc.vector.tensor_copy(
                                        hgaj[j][:, kp, :, 0:NCORES * w_j]
                                        .rearrange("p a (m w) -> p a m w",
                                                   w=w_j),
                                        hgs[:, :, 2 * kp:2 * kp + 2, 0:w_j]
                                        .rearrange("p m a w -> p a m w"))

                    # tail: last chunk compute + remaining finalizes
                    for g in range(16):
                        emit_p2_group(3, g)
                    finish_j_sums(3)
                    finalize_j(2)
                    finalize_j(3)

            pwo_cm.__exit__(None, None, None)

    nc.compile()
    return nc


def _t8(w, nk=8):
    m = w.shape[1]
    return np.ascontiguousarray(w.reshape(nk, P, m).transpose(1, 0, 2))


def _f8(x):
    return np.clip(x, -240.0, 240.0).astype(ml_dtypes.float8_e4m3)


def _prep_inputs(inputs):
    enc = np.asarray(inputs["encoder_outputs"], np.float32)
    ehid = np.asarray(inputs["encoder_hidden"], np.float32)
    targets = np.asarray(inputs["targets"])
    emb = np.asarray(inputs["emb"], np.float32)
    W1 = np.asarray(inputs["attn_W1"], np.float32)
    b1 = np.asarray(inputs["attn_b1"], np.float32)
    W2 = np.asarray(inputs["attn_W2"], np.float32)
    W_ih = np.asarray(inputs["W_ih"], np.float32)
    b_ih = np.asarray(inputs["b_ih"], np.float32)
    W_hh = np.asarray(inputs["W_hh"], np.float32)
    b_hh = np.asarray(inputs["b_hh"], np.float32)
    W_out = np.asarray(inputs["W_out"], np.float32)
    b_out = np.asarray(inputs["b_out"], np.float32)

    w1et8 = _f8(_t8(W1[:, :H].T) * SW)
    w1ht8 = _f8(_t8(np.ascontiguousarray(W1[:, H:]).T) * SW)
    wct8 = _f8(_t8(np.ascontiguousarray(W_ih[:, Dw:]).T) * SW)
    whht8 = _f8(_t8(W_hh.T) * SW)
    wxa = np.zeros((640, 3 * H), np.float32)
    wxa[:Dw] = W_ih[:, :Dw].T
    wxa[Dw] = b_ih + np.concatenate([b_hh[:2 * H], np.zeros(H, np.float32)])
    wxa8 = _f8(_t8(wxa, nk=5) * SX)
    w2t16 = np.ascontiguousarray(W2[0].reshape(KH, P).T).astype(np.float16)
    b1t = np.ascontiguousarray(b1.reshape(KH, P).T)
    bhnrep = np.ascontiguousarray(
        np.repeat(b_hh[2 * H:].reshape(KH, P).T[:, :, None], BC, axis=2)
        .reshape(P, KH * BC))

    x_all = emb[targets[:, :TS]]

    in_maps = []
    for c in range(NCORES):
        bsl = slice(c * BC, (c + 1) * BC)
        vsl = slice(c * VC, (c + 1) * VC)
        encT = np.zeros((H, BC, SP), np.float32)
        encT[:, :, :S] = enc[bsl].transpose(2, 0, 1)
        enct16 = _t8(encT.reshape(H, NBS)).astype(np.float16)
        xat = np.zeros((640, P), np.float32)
        xat[:Dw, :ROWS] = x_all[bsl].transpose(2, 1, 0).reshape(Dw, ROWS)
        xat[Dw, :ROWS] = 1.0
        xat16 = _t8(xat, nk=5).astype(np.float16)
        h0t = np.ascontiguousarray(
            ehid[0, bsl].T.reshape(KH, P, BC).transpose(1, 0, 2)
            .reshape(P, KH * BC)).astype(np.float16)
        wo8 = _f8(np.ascontiguousarray(
            (W_out[vsl].T * SW).reshape(KP, 2, P, VC).transpose(2, 0, 1, 3)))
        bout16 = np.ascontiguousarray(b_out[vsl][None, :]).astype(np.float16)
        in_maps.append({
            "enct16": enct16, "w1et8": w1et8, "wct8": wct8,
            "wxa8": wxa8, "xat16": xat16, "whht8": whht8, "w1ht8": w1ht8,
            "w2t16": w2t16, "b1t": b1t, "bhnrep": bhnrep, "h0t": h0t,
            "wo8": wo8, "bout16": bout16,
        })
    return in_maps


def kernel(**inputs):
    if "nc" not in _CACHE:
        _CACHE["nc"] = _build()
    nc = _CACHE["nc"]
    in_maps = _prep_inputs(inputs)
    res = run_bass_kernel_spmd(nc, in_maps, core_ids=list(range(NCORES)))
    L = np.stack([np.asarray(res.results[c]["out"], np.float32)
                  for c in range(NCORES)])
    L = (L.reshape(NCORES, NCORES, TS, BC, VC)
         .transpose(1, 3, 2, 0, 4).reshape(B, TS, V))
    return np.ascontiguousarray(L, dtype=np.float32)
